# revision 21
# baseline (speedup 1.0000x reference)
"""GaussianBlur2d Trainium2 kernel: 13x13 separable gaussian blur, reflect pad.

Input : x [32, 1, 1024, 1024] f32, kernel [1, 1, 13, 13] f32 (rank-1 separable).
Output: [32, 1, 1024, 1024] f32.

Strategy (pure data parallel, 4 images per core on 8 cores), all-bf16:
  The 2D conv is factored (SVD rank-1) into a vertical and a horizontal
  13-tap pass, both on the TensorEngine in bf16 (fp32 matmuls cost 4
  cycles/row on TRN2; bf16 costs 1, and the 2e-2 tolerance leaves bf16
  ~10x margin). The host converts x to bf16 (halves input DMA) and the
  kernel emits y^T in bf16 (halves output DMA; host untransposes).

  Pass 1 (vertical taps) keeps an IMAGE TILE stationary:
     t1[c, o] = sum_r X[r, c-window] * Bv[r, o]
  so the output arrives pre-transposed (partition=col) - exactly the
  contraction layout pass 2 needs. Windows are 128 rows at stride 116
  (6-row halo), so every output block is one matmul, no PSUM spills.
  Reflect-pad taps fold into the edge windows' band matrices.

  Pass 1 PSUM is a single [128, 1024] tile (2 banks); the one block
  crossing the col-512 bank boundary is split into two matmuls so no
  matmul write crosses a bank. That makes the PSUM->SBUF drain ONE
  copy per column group (DVE cost = free-size + fixed overhead, so
  fewer/larger copies win).

  Pass 2 (horizontal taps) keeps the BAND stationary:
     yT[o, r] = sum_c Bh[c, o] * t1[c, r]
  two N=512 matmuls per block stream the whole 1024-row extent, and
  one scalar-engine copy drains each [width, 1024] result to SBUF.
"""
import numpy as np
import ml_dtypes

import concourse.bacc as bacc
import concourse.mybir as mybir
import concourse.tile as tile
from concourse import bass_utils

F32 = mybir.dt.float32
BF16 = mybir.dt.bfloat16

H = 1024          # image rows/cols
SEG = 128         # stationary window height (contraction K)
KS = 13
HALF = KS // 2
N_CORES = 8
IMGS_PER_CORE = 4
BANK = 512        # PSUM bank width in f32 cols

# Output block widths are chosen for the HWDGE descriptor-split rule:
# a DMA is split across (largest divisor of partition count <= 16) SDMA
# engines, so 112-wide (16-way) and 120-wide (15-way) blocks spread output
# descriptors across all engines; 122/116/90 widths clump onto 2-4 engines.
# Interior windows: 112 outputs + 2*6 halo + 4 slack <= 128 rows. Edge
# windows are image-aligned (reflect taps fold into their bands).
BLOCK_STARTS = [0] + [120 + 112 * i for i in range(7)] + [904]
BLOCK_ENDS = [120] + [120 + 112 * (i + 1) for i in range(7)] + [1024]
NBLK = 9
# stationary window first row per block (clipped to the image)
WIN_STARTS = [0] + [120 + 112 * i - HALF for i in range(7)] + [H - SEG]


def _reflect(r):
    if r < 0:
        return -r
    if r > H - 1:
        return 2 * (H - 1) - r
    return r


def _decompose_kernel(k2d):
    k = np.asarray(k2d, dtype=np.float64).reshape(KS, KS)
    u, s, vh = np.linalg.svd(k)
    gv = u[:, 0] * np.sqrt(s[0])
    gh = vh[0, :] * np.sqrt(s[0])
    if gv.sum() < 0:
        gv, gh = -gv, -gh
    return gv, gh


def _build_bands(g):
    """Band matrix [128, 1024]: col o = taps of output o within its window."""
    out = np.zeros((SEG, H), dtype=np.float64)
    for blk in range(NBLK):
        o0, o1 = BLOCK_STARTS[blk], BLOCK_ENDS[blk]
        r0 = WIN_STARTS[blk]
        for o in range(o0, o1):
            for t in range(KS):
                rr = _reflect(o - HALF + t)
                if r0 <= rr < r0 + SEG:
                    out[rr - r0, o] += g[t]
    return out.astype(ml_dtypes.bfloat16)


def _pass1_segments():
    """(blk, s0, s1) matmul segments, no segment crossing the bank edge."""
    segs = []
    for blk in range(NBLK):
        o0, o1 = BLOCK_STARTS[blk], BLOCK_ENDS[blk]
        if o0 < BANK < o1:
            segs.append((blk, o0, BANK))
            segs.append((blk, BANK, o1))
        else:
            segs.append((blk, o0, o1))
    return segs


_SEGS = _pass1_segments()
N_WARM = 12  # HAM warmup matmuls issued under the initial input DMA


def _build_program(shared_bands):
    # shared_bands: separable factors equal (symmetric kernel) -> one band
    # array serves both passes
    nbc = H if shared_bands else 2 * H
    p2off = 0 if shared_bands else H
    nc = bacc.Bacc("TRN2", target_bir_lowering=False, debug=False)
    x = nc.dram_tensor("x", [H, IMGS_PER_CORE, H], BF16, kind="ExternalInput")
    bands = nc.dram_tensor("bands", [SEG, nbc], BF16, kind="ExternalInput")
    y = nc.dram_tensor("y", [IMGS_PER_CORE, H, H], BF16, kind="ExternalOutput")

    with tile.TileContext(nc) as tc:
        with (
            tc.tile_pool(name="xp", bufs=1) as xp,
            tc.tile_pool(name="t1p", bufs=4) as t1p,
            tc.tile_pool(name="op", bufs=8) as op,
            tc.tile_pool(name="bp", bufs=1) as bp,
            tc.tile_pool(name="ps", bufs=2, space="PSUM") as psp,
        ):
            bt = bp.tile([SEG, nbc], BF16, tag="bands")
            nc.sync.dma_start(bt[:], bands[:])

            # HAM warmup: junk matmuls on the band tile keep the PE busy
            # during the first image's input DMA so real matmuls run at
            # the warm 2.4 GHz clock.
            wps = psp.tile([SEG, H], F32, name="warm", tag="ph")
            for i in range(N_WARM):
                half = (i % 2) * BANK
                nc.tensor.matmul(
                    wps[:, half:half + BANK], bt[:, 0:SEG], bt[:, 0:BANK],
                    start=(i < 2), stop=(i >= N_WARM - 2),
                )

            def pass2(b, cg, t1):
                # pass 2 for output col block cg: band stationary, t1
                # moving; output y^T block [width, 1024]
                o0 = BLOCK_STARTS[cg]
                w = BLOCK_ENDS[cg] - o0
                ph = psp.tile([SEG, H], F32, name=f"psh{cg}", tag="ph")
                for half in (0, BANK):
                    nc.tensor.matmul(
                        ph[:w, half:half + BANK],
                        bt[:, p2off + o0:p2off + o0 + w],
                        t1[:, half:half + BANK],
                        start=True, stop=True,
                    )
                yt = op.tile([SEG, H], BF16, name=f"yt{cg}", tag="yt")
                nc.scalar.copy(yt[:w, :], ph[:w, :])
                nc.sync.dma_start(y[b, o0:o0 + w, :], yt[:w, :])

            # Input DMAs on the GpSimd SWDGE ring (its own descriptor
            # engine; HWDGE sequencers stay free for outputs/copies).
            # x is [row, img, col] host-side so images 1-3 load as 6KB
            # DRAM lines (3 imgs x 2KB contiguous per row) - fewer, fatter
            # descriptors. Image 0 loads first as its own 9 small DMAs
            # (ring FIFO = strict priority) so compute starts after
            # 2.4 MB, not 9.4 MB. All input DMAs issue up-front; SBUF
            # holds the whole core's input (72 KB/partition).
            x0w = []
            xbw = []
            for blk in range(NBLK):
                r0 = WIN_STARTS[blk]
                xs = xp.tile([SEG, H], BF16, name=f"x0t{blk}", tag=f"x{blk}")
                nc.gpsimd.dma_start(xs[:], x[r0:r0 + SEG, 0, :])
                x0w.append(xs)
            for blk in range(NBLK):
                r0 = WIN_STARTS[blk]
                xs = xp.tile([SEG, (IMGS_PER_CORE - 1) * H], BF16,
                             name=f"xbt{blk}", tag=f"g{blk}")
                nc.gpsimd.dma_start(xs[:], x[r0:r0 + SEG, 1:IMGS_PER_CORE, :])
                xbw.append(xs)

            pending = None  # (b, cg, t1) whose pass 2 hasn't been issued
            for b in range(IMGS_PER_CORE):
                # pass 1: vertical taps; col-group cg covers image cols
                # [WIN_STARTS[cg], +128); output t1 = T1^T group [col, row].
                # Pass 2 of group g-1 is issued after pass 1 of group g so
                # the PE never waits on the PSUM->SBUF cast of its own group
                # (software pipelining by one group).
                if b == 0:
                    xts = [(x0w[blk], 0) for blk in range(NBLK)]
                else:
                    xts = [(xbw[blk], (b - 1) * H) for blk in range(NBLK)]
                for cg in range(NBLK):
                    c0 = WIN_STARTS[cg]
                    ps = psp.tile([SEG, H], F32, name=f"psv{cg}", tag="pv")
                    b0 = b1 = True  # per-bank start flag
                    for (blk, s0, s1) in _SEGS:
                        st = b0 if s0 < BANK else b1
                        if s0 < BANK:
                            b0 = False
                        else:
                            b1 = False
                        xw, xoff = xts[blk]
                        nc.tensor.matmul(
                            ps[:, s0:s1],
                            xw[:, xoff + c0:xoff + c0 + SEG],
                            bt[:, s0:s1],
                            start=st,
                            stop=(s1 == BANK or s1 == H),
                        )
                    t1 = t1p.tile([SEG, H], BF16, name=f"t1{cg}", tag="t1")
                    nc.vector.tensor_copy(t1[:], ps[:])
                    if pending is not None:
                        pass2(*pending)
                    pending = (b, cg, t1)
            pass2(*pending)
    nc.compile()
    return nc


_NC_CACHE = {}


def _get_program(shared_bands):
    if shared_bands not in _NC_CACHE:
        _NC_CACHE[shared_bands] = _build_program(shared_bands)
    return _NC_CACHE[shared_bands]


def run(x, kernel, trace=False, tmpdir=None):
    """Full-input entry. Returns (y, BassKernelResults)."""
    x = np.asarray(x, dtype=np.float32).reshape(32, H, H)
    xb = np.ascontiguousarray(x).astype(ml_dtypes.bfloat16)
    gv, gh = _decompose_kernel(kernel)
    shared = bool(np.allclose(gv, gh, rtol=0, atol=1e-12 * np.abs(gv).max()))
    if shared:
        bands = _build_bands(gv)
    else:
        bands = np.concatenate([_build_bands(gv), _build_bands(gh)], axis=1)
    nc = _get_program(shared)
    in_maps = [
        {"x": np.ascontiguousarray(
            xb[c * IMGS_PER_CORE:(c + 1) * IMGS_PER_CORE].transpose(1, 0, 2)),
         "bands": bands}
        for c in range(N_CORES)
    ]
    res = bass_utils.run_bass_kernel_spmd(
        nc, in_maps, core_ids=list(range(N_CORES)), trace=trace, tmpdir=tmpdir)
    yt = np.concatenate([np.asarray(res.results[c]["y"]) for c in range(N_CORES)],
                        axis=0)
    y = np.ascontiguousarray(yt.transpose(0, 2, 1)).astype(np.float32)
    return y.reshape(32, 1, H, H), res


def kernel(x, kernel):
    y, _ = run(x, kernel, trace=False)
    return y


# revision 22
# speedup vs baseline: 1.0439x; 1.0439x over previous
"""GaussianBlur2d Trainium2 kernel: 13x13 separable gaussian blur, reflect pad.

Input : x [32, 1, 1024, 1024] f32, kernel [1, 1, 13, 13] f32 (rank-1 separable).
Output: [32, 1, 1024, 1024] f32.

Strategy (pure data parallel, 4 images per core on 8 cores), all-bf16:
  The 2D conv is factored (SVD rank-1) into a vertical and a horizontal
  13-tap pass, both on the TensorEngine in bf16 (fp32 matmuls cost 4
  cycles/row on TRN2; bf16 costs 1, and the 2e-2 tolerance leaves bf16
  ~10x margin). The host converts x to bf16 (halves input DMA) and the
  kernel emits y^T in bf16 (halves output DMA; host untransposes).

  Pass 1 (vertical taps) keeps an IMAGE TILE stationary:
     t1[c, o] = sum_r X[r, c-window] * Bv[r, o]
  so the output arrives pre-transposed (partition=col) - exactly the
  contraction layout pass 2 needs. Windows are 128 rows at stride 112
  (6-row halo); every 112/120-wide output block is computed by single
  matmuls with no cross-window PSUM accumulation, and reflect-pad taps
  fold into the edge windows' band matrices.

  Pass 1 PSUM is a single [128, 1024] tile (2 banks); the one block
  crossing the col-512 bank boundary is split into two matmuls so no
  matmul write crosses a bank. The PSUM->SBUF drain is then ONE vector
  copy per column group (copy cost = free-size + fixed overhead).

  Pass 2 (horizontal taps) keeps the BAND stationary:
     yT[o, r] = sum_c Bh[c, o] * t1[c, r]
  two N=512 matmuls per block stream the whole 1024-row extent, and one
  scalar-engine copy drains each [width, 1024] result to SBUF.

  Scheduling: pass 2 of group g-1 issues after pass 1 of group g
  (the PE never waits on its own group's PSUM drain); input window DMAs
  ride the GpSimd SWDGE ring, staggered one window per group; output
  DMAs ride the sync HWDGE ring; PSUM drains split Vector (pass 1) /
  Scalar (pass 2). Output block widths 120/112 are chosen for the HWDGE
  descriptor-split rule (largest divisor of partition count <= 16 SDMA
  engines), so output descriptors spread across 15-16 engines instead
  of clumping onto 2-4.
"""
import numpy as np
import ml_dtypes

import concourse.bacc as bacc
import concourse.mybir as mybir
import concourse.tile as tile
from concourse import bass_utils

F32 = mybir.dt.float32
BF16 = mybir.dt.bfloat16

H = 1024          # image rows/cols
SEG = 128         # stationary window height (contraction K)
KS = 13
HALF = KS // 2
N_CORES = 8
IMGS_PER_CORE = 4
BANK = 512        # PSUM bank width in f32 cols

BLOCK_STARTS = [0] + [120 + 112 * i for i in range(7)] + [904]
BLOCK_ENDS = [120] + [120 + 112 * (i + 1) for i in range(7)] + [1024]
NBLK = 9
# stationary window first row per block (clipped to the image)
WIN_STARTS = [0] + [120 + 112 * i - HALF for i in range(7)] + [H - SEG]


def _reflect(r):
    if r < 0:
        return -r
    if r > H - 1:
        return 2 * (H - 1) - r
    return r


def _decompose_kernel(k2d):
    k = np.asarray(k2d, dtype=np.float64).reshape(KS, KS)
    u, s, vh = np.linalg.svd(k)
    gv = u[:, 0] * np.sqrt(s[0])
    gh = vh[0, :] * np.sqrt(s[0])
    if gv.sum() < 0:
        gv, gh = -gv, -gh
    return gv, gh


def _build_bands(g):
    """Band matrix [128, 1024]: col o = taps of output o within its window."""
    out = np.zeros((SEG, H), dtype=np.float64)
    for blk in range(NBLK):
        o0, o1 = BLOCK_STARTS[blk], BLOCK_ENDS[blk]
        r0 = WIN_STARTS[blk]
        for o in range(o0, o1):
            for t in range(KS):
                rr = _reflect(o - HALF + t)
                if r0 <= rr < r0 + SEG:
                    out[rr - r0, o] += g[t]
    return out.astype(ml_dtypes.bfloat16)


def _pass1_segments():
    """(blk, s0, s1) matmul segments, no segment crossing the bank edge."""
    segs = []
    for blk in range(NBLK):
        o0, o1 = BLOCK_STARTS[blk], BLOCK_ENDS[blk]
        if o0 < BANK < o1:
            segs.append((blk, o0, BANK))
            segs.append((blk, BANK, o1))
        else:
            segs.append((blk, o0, o1))
    return segs


_SEGS = _pass1_segments()
N_WARM = 12  # HAM warmup matmuls issued under the initial input DMA


def _build_program(shared_bands):
    # shared_bands: separable factors equal (symmetric kernel) -> one band
    # array serves both passes
    nbc = H if shared_bands else 2 * H
    p2off = 0 if shared_bands else H
    nc = bacc.Bacc("TRN2", target_bir_lowering=False, debug=False)
    x = nc.dram_tensor("x", [IMGS_PER_CORE, H, H], BF16, kind="ExternalInput")
    bands = nc.dram_tensor("bands", [SEG, nbc], BF16, kind="ExternalInput")
    y = nc.dram_tensor("y", [IMGS_PER_CORE, H, H], BF16, kind="ExternalOutput")

    with tile.TileContext(nc) as tc:
        with (
            tc.tile_pool(name="xp", bufs=2) as xp,
            tc.tile_pool(name="t1p", bufs=4) as t1p,
            tc.tile_pool(name="op", bufs=8) as op,
            tc.tile_pool(name="bp", bufs=1) as bp,
            tc.tile_pool(name="ps", bufs=2, space="PSUM") as psp,
        ):
            bt = bp.tile([SEG, nbc], BF16, tag="bands")
            nc.sync.dma_start(bt[:], bands[:])

            # HAM warmup: junk matmuls on the band tile keep the PE busy
            # during the first image's input DMA so real matmuls run at
            # the warm 2.4 GHz clock.
            wps = psp.tile([SEG, H], F32, name="warm", tag="ph")
            for i in range(N_WARM):
                half = (i % 2) * BANK
                nc.tensor.matmul(
                    wps[:, half:half + BANK], bt[:, 0:SEG], bt[:, 0:BANK],
                    start=(i < 2), stop=(i >= N_WARM - 2),
                )

            def pass2(b, cg, t1):
                # pass 2 for output col block cg: band stationary, t1
                # moving; output y^T block [width, 1024]
                o0 = BLOCK_STARTS[cg]
                w = BLOCK_ENDS[cg] - o0
                ph = psp.tile([SEG, H], F32, name=f"psh{cg}", tag="ph")
                for half in (0, BANK):
                    nc.tensor.matmul(
                        ph[:w, half:half + BANK],
                        bt[:, p2off + o0:p2off + o0 + w],
                        t1[:, half:half + BANK],
                        start=True, stop=True,
                    )
                yt = op.tile([SEG, H], BF16, name=f"yt{cg}", tag="yt")
                nc.scalar.copy(yt[:w, :], ph[:w, :])
                nc.sync.dma_start(y[b, o0:o0 + w, :], yt[:w, :])

            xtiles = {}

            def load_window(b, blk, split=False):
                # input DMAs ride the GpSimd SWDGE ring: its Q7
                # CounterMachine emits descriptors while both HWDGE
                # sequencers stay free (sync: outputs, scalar: drains).
                # split=True loads the window in column halves so image
                # 0's left-half groups can start after ~1.2 MB arrives.
                r0 = WIN_STARTS[blk]
                xs = xp.tile([SEG, H], BF16, name=f"xt{blk}", tag=f"x{blk}")
                if split:
                    nc.gpsimd.dma_start(xs[:, 0:BANK], x[b, r0:r0 + SEG, 0:BANK])
                else:
                    nc.gpsimd.dma_start(xs[:], x[b, r0:r0 + SEG, :])
                xtiles[(b, blk)] = xs
                return xs

            right = []
            for blk in range(NBLK):
                right.append(load_window(0, blk, split=True))
            for blk in range(NBLK):
                r0 = WIN_STARTS[blk]
                nc.gpsimd.dma_start(right[blk][:, BANK:H],
                                    x[0, r0:r0 + SEG, BANK:H])

            pending = None  # (b, cg, t1) whose pass 2 hasn't been issued
            for b in range(IMGS_PER_CORE):
                xts = [xtiles[(b, blk)] for blk in range(NBLK)]
                # pass 1: vertical taps; col-group cg covers image cols
                # [WIN_STARTS[cg], +128); output t1 = T1^T group [col, row].
                # Pass 2 of group g-1 issues after pass 1 of group g so
                # the PE never waits on its own group's PSUM drain; the
                # next image's windows are prefetched one per group.
                for cg in range(NBLK):
                    if b + 1 < IMGS_PER_CORE:
                        load_window(b + 1, cg)
                    c0 = WIN_STARTS[cg]
                    ps = psp.tile([SEG, H], F32, name=f"psv{cg}", tag="pv")
                    b0 = b1 = True  # per-bank start flag
                    for (blk, s0, s1) in _SEGS:
                        st = b0 if s0 < BANK else b1
                        if s0 < BANK:
                            b0 = False
                        else:
                            b1 = False
                        nc.tensor.matmul(
                            ps[:, s0:s1],
                            xts[blk][:, c0:c0 + SEG],
                            bt[:, s0:s1],
                            start=st,
                            stop=(s1 == BANK or s1 == H),
                        )
                    t1 = t1p.tile([SEG, H], BF16, name=f"t1{cg}", tag="t1")
                    nc.vector.tensor_copy(t1[:], ps[:])
                    if pending is not None:
                        pass2(*pending)
                    pending = (b, cg, t1)
            pass2(*pending)
    nc.compile()
    return nc


_NC_CACHE = {}


def _get_program(shared_bands):
    if shared_bands not in _NC_CACHE:
        _NC_CACHE[shared_bands] = _build_program(shared_bands)
    return _NC_CACHE[shared_bands]


def run(x, kernel, trace=False, tmpdir=None):
    """Full-input entry. Returns (y, BassKernelResults)."""
    x = np.asarray(x, dtype=np.float32).reshape(32, H, H)
    xb = np.ascontiguousarray(x).astype(ml_dtypes.bfloat16)
    gv, gh = _decompose_kernel(kernel)
    shared = bool(np.allclose(gv, gh, rtol=0, atol=1e-12 * np.abs(gv).max()))
    if shared:
        bands = _build_bands(gv)
    else:
        bands = np.concatenate([_build_bands(gv), _build_bands(gh)], axis=1)
    nc = _get_program(shared)
    in_maps = [
        {"x": xb[c * IMGS_PER_CORE:(c + 1) * IMGS_PER_CORE], "bands": bands}
        for c in range(N_CORES)
    ]
    res = bass_utils.run_bass_kernel_spmd(
        nc, in_maps, core_ids=list(range(N_CORES)), trace=trace, tmpdir=tmpdir)
    yt = np.concatenate([np.asarray(res.results[c]["y"]) for c in range(N_CORES)],
                        axis=0)
    y = np.ascontiguousarray(yt.transpose(0, 2, 1)).astype(np.float32)
    return y.reshape(32, 1, H, H), res


def kernel(x, kernel):
    y, _ = run(x, kernel, trace=False)
    return y


# revision 23
# speedup vs baseline: 1.1565x; 1.1079x over previous
"""GaussianBlur2d Trainium2 kernel: 13x13 separable gaussian blur, reflect pad.

Input : x [32, 1, 1024, 1024] f32, kernel [1, 1, 13, 13] f32 (rank-1 separable).
Output: [32, 1, 1024, 1024] f32.

Strategy (pure data parallel, 4 images per core on 8 cores), all-bf16:
  The 2D conv is factored (SVD rank-1) into a vertical and a horizontal
  13-tap pass, both on the TensorEngine in bf16 (fp32 matmuls cost 4
  cycles/row on TRN2; bf16 costs 1, and the 2e-2 tolerance leaves bf16
  ~10x margin). The host converts x to bf16 (halves input DMA) and the
  kernel emits y^T in bf16 (halves output DMA; host untransposes).

  Pass 1 (vertical taps) keeps an IMAGE TILE stationary:
     t1[c, o] = sum_r X[r, c-window] * Bv[r, o]
  so the output arrives pre-transposed (partition=col) - exactly the
  contraction layout pass 2 needs. Windows are 128 rows at stride 112
  (6-row halo); every 112/120-wide output block is computed by single
  matmuls with no cross-window PSUM accumulation, and reflect-pad taps
  fold into the edge windows' band matrices.

  Pass 1 PSUM is a single [128, 1024] tile (2 banks); the one block
  crossing the col-512 bank boundary is split into two matmuls so no
  matmul write crosses a bank. The PSUM->SBUF drain is then ONE vector
  copy per column group (copy cost = free-size + fixed overhead).

  Pass 2 (horizontal taps) keeps the BAND stationary:
     yT[o, r] = sum_c Bh[c, o] * t1[c, r]
  two N=512 matmuls per block stream the whole 1024-row extent, and one
  scalar-engine copy drains each [width, 1024] result to SBUF.

  Scheduling: pass 2 of group g-1 issues after pass 1 of group g
  (the PE never waits on its own group's PSUM drain); input window DMAs
  ride the GpSimd SWDGE ring, staggered one window per group; output
  DMAs ride the sync HWDGE ring; PSUM drains split Vector (pass 1) /
  Scalar (pass 2). Output block widths 120/112 are chosen for the HWDGE
  descriptor-split rule (largest divisor of partition count <= 16 SDMA
  engines), so output descriptors spread across 15-16 engines instead
  of clumping onto 2-4.
"""
import numpy as np
import ml_dtypes

import concourse.bacc as bacc
import concourse.mybir as mybir
import concourse.tile as tile
from concourse import bass_utils

F32 = mybir.dt.float32
BF16 = mybir.dt.bfloat16

H = 1024          # image rows/cols
SEG = 128         # stationary window height (contraction K)
KS = 13
HALF = KS // 2
N_CORES = 8
IMGS_PER_CORE = 4
BANK = 512        # PSUM bank width in f32 cols

BLOCK_STARTS = [0] + [120 + 112 * i for i in range(7)] + [904]
BLOCK_ENDS = [120] + [120 + 112 * (i + 1) for i in range(7)] + [1024]
NBLK = 9
# stationary window first row per block (clipped to the image)
WIN_STARTS = [0] + [120 + 112 * i - HALF for i in range(7)] + [H - SEG]


def _reflect(r):
    if r < 0:
        return -r
    if r > H - 1:
        return 2 * (H - 1) - r
    return r


def _decompose_kernel(k2d):
    k = np.asarray(k2d, dtype=np.float64).reshape(KS, KS)
    u, s, vh = np.linalg.svd(k)
    gv = u[:, 0] * np.sqrt(s[0])
    gh = vh[0, :] * np.sqrt(s[0])
    if gv.sum() < 0:
        gv, gh = -gv, -gh
    return gv, gh


def _build_bands(g):
    """Band matrix [128, 1024]: col o = taps of output o within its window."""
    out = np.zeros((SEG, H), dtype=np.float64)
    for blk in range(NBLK):
        o0, o1 = BLOCK_STARTS[blk], BLOCK_ENDS[blk]
        r0 = WIN_STARTS[blk]
        for o in range(o0, o1):
            for t in range(KS):
                rr = _reflect(o - HALF + t)
                if r0 <= rr < r0 + SEG:
                    out[rr - r0, o] += g[t]
    return out.astype(ml_dtypes.bfloat16)


def _pass1_segments():
    """(blk, s0, s1) matmul segments, no segment crossing the bank edge."""
    segs = []
    for blk in range(NBLK):
        o0, o1 = BLOCK_STARTS[blk], BLOCK_ENDS[blk]
        if o0 < BANK < o1:
            segs.append((blk, o0, BANK))
            segs.append((blk, BANK, o1))
        else:
            segs.append((blk, o0, o1))
    return segs


_SEGS = _pass1_segments()
N_WARM = 12  # HAM warmup matmuls issued under the initial input DMA


def _build_program(shared_bands):
    # shared_bands: separable factors equal (symmetric kernel) -> one band
    # array serves both passes
    nbc = H if shared_bands else 2 * H
    p2off = 0 if shared_bands else H
    nc = bacc.Bacc("TRN2", target_bir_lowering=False, debug=False)
    x = nc.dram_tensor("x", [IMGS_PER_CORE, H, H], BF16, kind="ExternalInput")
    bands = nc.dram_tensor("bands", [SEG, nbc], BF16, kind="ExternalInput")
    y = nc.dram_tensor("y", [IMGS_PER_CORE, H, H], BF16, kind="ExternalOutput")

    with tile.TileContext(nc) as tc:
        with (
            tc.tile_pool(name="xp", bufs=2) as xp,
            tc.tile_pool(name="t1p", bufs=4) as t1p,
            tc.tile_pool(name="op", bufs=8) as op,
            tc.tile_pool(name="bp", bufs=1) as bp,
            tc.tile_pool(name="ps", bufs=2, space="PSUM") as psp,
        ):
            bt = bp.tile([SEG, nbc], BF16, tag="bands")
            nc.sync.dma_start(bt[:], bands[:])

            # HAM warmup: junk matmuls on the band tile keep the PE busy
            # during the first image's input DMA so real matmuls run at
            # the warm 2.4 GHz clock.
            wps = psp.tile([SEG, H], F32, name="warm", tag="ph")
            for i in range(N_WARM):
                half = (i % 2) * BANK
                nc.tensor.matmul(
                    wps[:, half:half + BANK], bt[:, 0:SEG], bt[:, 0:BANK],
                    start=(i < 2), stop=(i >= N_WARM - 2),
                )

            def pass2(b, cg, t1):
                # pass 2 for output col block cg: band stationary, t1
                # moving; output y^T block [width, 1024]
                o0 = BLOCK_STARTS[cg]
                w = BLOCK_ENDS[cg] - o0
                ph = psp.tile([SEG, H], F32, name=f"psh{cg}", tag="ph")
                for half in (0, BANK):
                    nc.tensor.matmul(
                        ph[:w, half:half + BANK],
                        bt[:, p2off + o0:p2off + o0 + w],
                        t1[:, half:half + BANK],
                        start=True, stop=True,
                    )
                yt = op.tile([SEG, H], BF16, name=f"yt{cg}", tag="yt")
                nc.scalar.copy(yt[:w, :], ph[:w, :])
                nc.sync.dma_start(y[b, o0:o0 + w, :], yt[:w, :])

            xtiles = {}

            def load_window(b, blk, split=False):
                # input DMAs ride the GpSimd SWDGE ring: its Q7
                # CounterMachine emits descriptors while both HWDGE
                # sequencers stay free (sync: outputs, scalar: drains).
                # split=True loads the window in column halves so image
                # 0's left-half groups can start after ~1.2 MB arrives.
                r0 = WIN_STARTS[blk]
                xs = xp.tile([SEG, H], BF16, name=f"xt{blk}", tag=f"x{blk}")
                if split:
                    nc.gpsimd.dma_start(xs[:, 0:BANK], x[b, r0:r0 + SEG, 0:BANK])
                else:
                    nc.gpsimd.dma_start(xs[:], x[b, r0:r0 + SEG, :])
                xtiles[(b, blk)] = xs
                return xs

            for blk in range(NBLK):
                load_window(0, blk)

            pending = None  # (b, cg, t1) whose pass 2 hasn't been issued
            for b in range(IMGS_PER_CORE):
                xts = [xtiles[(b, blk)] for blk in range(NBLK)]
                # pass 1: vertical taps; col-group cg covers image cols
                # [WIN_STARTS[cg], +128); output t1 = T1^T group [col, row].
                # Pass 2 of group g-1 issues after pass 1 of group g so
                # the PE never waits on its own group's PSUM drain; the
                # next image's windows are prefetched one per group.
                for cg in range(NBLK):
                    if b + 1 < IMGS_PER_CORE:
                        load_window(b + 1, cg)
                    c0 = WIN_STARTS[cg]
                    ps = psp.tile([SEG, H], F32, name=f"psv{cg}", tag="pv")
                    b0 = b1 = True  # per-bank start flag
                    for (blk, s0, s1) in _SEGS:
                        st = b0 if s0 < BANK else b1
                        if s0 < BANK:
                            b0 = False
                        else:
                            b1 = False
                        nc.tensor.matmul(
                            ps[:, s0:s1],
                            xts[blk][:, c0:c0 + SEG],
                            bt[:, s0:s1],
                            start=st,
                            stop=(s1 == BANK or s1 == H),
                        )
                    t1 = t1p.tile([SEG, H], BF16, name=f"t1{cg}", tag="t1")
                    nc.vector.tensor_copy(t1[:], ps[:])
                    if pending is not None:
                        pass2(*pending)
                    pending = (b, cg, t1)
            pass2(*pending)
    nc.compile()
    return nc


_NC_CACHE = {}


def _get_program(shared_bands):
    if shared_bands not in _NC_CACHE:
        _NC_CACHE[shared_bands] = _build_program(shared_bands)
    return _NC_CACHE[shared_bands]


def run(x, kernel, trace=False, tmpdir=None):
    """Full-input entry. Returns (y, BassKernelResults)."""
    x = np.asarray(x, dtype=np.float32).reshape(32, H, H)
    xb = np.ascontiguousarray(x).astype(ml_dtypes.bfloat16)
    gv, gh = _decompose_kernel(kernel)
    shared = bool(np.allclose(gv, gh, rtol=0, atol=1e-12 * np.abs(gv).max()))
    if shared:
        bands = _build_bands(gv)
    else:
        bands = np.concatenate([_build_bands(gv), _build_bands(gh)], axis=1)
    nc = _get_program(shared)
    in_maps = [
        {"x": xb[c * IMGS_PER_CORE:(c + 1) * IMGS_PER_CORE], "bands": bands}
        for c in range(N_CORES)
    ]
    res = bass_utils.run_bass_kernel_spmd(
        nc, in_maps, core_ids=list(range(N_CORES)), trace=trace, tmpdir=tmpdir)
    yt = np.concatenate([np.asarray(res.results[c]["y"]) for c in range(N_CORES)],
                        axis=0)
    y = np.ascontiguousarray(yt.transpose(0, 2, 1)).astype(np.float32)
    return y.reshape(32, 1, H, H), res


def kernel(x, kernel):
    y, _ = run(x, kernel, trace=False)
    return y
